# revision 21
# baseline (speedup 1.0000x reference)
"""Trainium2 Bass kernel for nn_AssociativeMemoryStep (forward-looking retention).

reference semantics:
    q,k,v,o weights = basis @ {q,k,v,o}_coeffs.T          [V, C]
    q/k/v = x @ w                                         [B, T, C]
    scores[t,s] = (q_t . k_s) * decay^(s-t-1) for s>t     (anti-causal)
    retrieved = scores @ v ; out = retrieved @ o_w.T * out_scale

Factored (basis-space) formulation: all four weights share the factor
`basis` [V, 2NB], so with xb = x @ basis [T, 2NB]:
    scores    = (xb @ M) @ xb^T          M  = q_coeffs^T @ k_coeffs  [2NB,2NB]
    out       = (scores_decayed @ xb) @ Wo'
    Wo'       = v_coeffs^T @ oc @ basis^T * out_scale               [2NB, V]
i.e. retention runs with q := xb@M, k := v := xb.

BANDED retention: decay = sigmoid(3) ~ 0.9526, so contributions from
s - t > 256 are < 4e-6.  Each 128-wide t-tile i attends only s-blocks
j = i (d0, strict lower-tri decay mask) and j = i+1 (d1, full decay
mask); truncation rel-l2 ~ 6e-4.  No recurrent state, no cross-chunk
terms -- all tiles fully independent, which lets retention + output
projection interleave with the DMA-paced projection phase.

Sharding: 8 cores = 4 batches x 2 sequence halves of T_LOC=2048, each
with a HALO=128 slice of the following positions recomputed locally.

Schedule: reverse pch (512-col) order; iter p emits
  inproj(p) -> qproj(p) -> outproj(p+1) -> transposes(p) ->
  [halo @ first iter] -> A-blocks(p) -> intra(p)
(outproj of the previous chunk hides the kT/qT evacuation latency; the
last iter defers outproj(1) behind intra(0) to hide the rT(0) evacs
before the epilogue outproj(0)).

Measured constraints this schedule is built around:
- HAM clock gate: PE runs at 1.2GHz until ~3.4us of SUSTAINED matmul
  activity; transpose-mode does NOT count.  Warmup = real matmuls on a
  memset tile (no DMA dependency, starts right after the preamble) and
  fillers pad the DMA-paced first inproj so the window never breaks.
- DMA: issues start only after the ~6.5us framework preamble; first
  bytes land ~8.3us; ~300-360GB/s once ramped.  Input is 5.7MB, so the
  head of the stream carries exactly what the first matmuls need (each
  kt-pair's weights ahead of its x piece, on the same ring; big x/og
  traffic on sync+gpsimd, small constants on the scalar ring), and
  every piece is a fully contiguous DRAM block (host pre-tiled).
- PSUM evacuation is ~690ns per [128,512] fp32 bank on either ACT or
  DVE (1x mode): evacuations alternate engines everywhere, and outproj
  og evacuations alternate per-vt so neither engine falls behind the
  0.43us/psum matmul pace.
- Each HWDGE store pays ~1.5-2us completion latency: the final output
  goes out as 2 large 512KB stores (one per HWDGE ring), not 8 small.
"""

import contextlib

import numpy as np
import ml_dtypes

import concourse.bass as bass
import concourse.mybir as mybir
import concourse.tile as tile
from concourse import bacc
from concourse.bass_utils import run_bass_kernel_spmd

BF16 = ml_dtypes.bfloat16

B, T, V, C = 4, 4096, 1024, 256   # C = 2*N_BASIS = basis channels
N_CORES = 8
T_LOC = 2048          # main positions per core
HALO = 128            # halo positions beyond T_LOC (d1 context for tile 15)
PCH = 512             # projection t-chunk
N_PCH = T_LOC // PCH  # 4
KT = V // 128         # 8 v-ktiles
CT = C // 128         # 2 c-tiles
TPP = PCH // 128      # 4 t-tiles per pch
N_WARM = 52           # warmup matmuls bridging [preamble-end, x3p0 ~12.5us];
                      # ~107ns each at the cold 1.2GHz clock

FP32 = mybir.dt.float32
BF = mybir.dt.bfloat16


def build_nc():
    nc = bacc.Bacc("TRN2", target_bir_lowering=False, debug=False,
                   num_devices=N_CORES)

    # all inputs host-pre-tiled to be contiguous per [128, N] DMA block
    xh_d = nc.dram_tensor("xh", [N_PCH * 4 * 128, 1024], BF, kind="ExternalInput")
    xhh_d = nc.dram_tensor("xhh", [128, KT * HALO], BF, kind="ExternalInput")
    wb_d = nc.dram_tensor("wb", [4 * 128, 2 * C], BF, kind="ExternalInput")
    mt_d = nc.dram_tensor("mt", [128, CT * C], BF, kind="ExternalInput")
    owT_d = nc.dram_tensor("owT", [2 * 128, V], BF, kind="ExternalInput")
    maskC_d = nc.dram_tensor("maskC", [128, 2 * 128], FP32, kind="ExternalInput")
    ident_d = nc.dram_tensor("ident", [128, 128], BF, kind="ExternalInput")
    outT_d = nc.dram_tensor("outT", [V, T_LOC], BF, kind="ExternalOutput")

    with tile.TileContext(nc) as tc:
        build_tile(tc, xh_d, xhh_d, wb_d, mt_d, owT_d, maskC_d, ident_d, outT_d)
    nc.compile()
    return nc


def build_tile(tc, xh_d, xhh_d, wb_d, mt_d, owT_d, maskC_d, ident_d, outT_d):
    nc = tc.nc

    ctx = contextlib.ExitStack()
    consts = ctx.enter_context(tc.tile_pool(name="consts", bufs=1))
    xpool = ctx.enter_context(tc.tile_pool(name="xpool", bufs=4))
    kpool = ctx.enter_context(tc.tile_pool(name="kpool", bufs=2))
    qpool = ctx.enter_context(tc.tile_pool(name="qpool", bufs=2))
    vpool = ctx.enter_context(tc.tile_pool(name="vpool", bufs=2))
    rpool = ctx.enter_context(tc.tile_pool(name="rpool", bufs=2))
    apool = ctx.enter_context(tc.tile_pool(name="apool", bufs=6))
    ostage = ctx.enter_context(tc.tile_pool(name="ostage", bufs=4))
    psA = ctx.enter_context(tc.tile_pool(name="psA", bufs=2, space="PSUM"))
    psO = ctx.enter_context(tc.tile_pool(name="psO", bufs=3, space="PSUM"))
    psT = ctx.enter_context(tc.tile_pool(name="psT", bufs=3, space="PSUM"))

    # ---- constant tiles ----
    wbA_sb = consts.tile([128, 4, C], BF)
    wbB_sb = consts.tile([128, 4, C], BF)
    mt_sb = consts.tile([128, CT, C], BF)
    owT_sb = consts.tile([128, CT * V], BF)
    maskC_sb = consts.tile([128, 2 * 128], FP32)   # [d1 mask | d0 tri mask]
    ident_sb = consts.tile([128, 128], BF)
    kTh_sb = consts.tile([128, CT, HALO], BF)      # halo xbT
    vh_sb = consts.tile([128, C], BF)              # halo xb in [t, c]
    xth = consts.tile([128, KT * HALO], BF)        # halo x staging

    # ---- startup DMAs; HWDGE(sync) + SWDGE(gpsimd) queues run concurrently,
    # pieces ordered so wb[kt]+x3[kt] needed by the first matmuls land first
    xt3 = xpool.tile([128, KT * PCH], BF, tag="xt", name="x3")

    def xpiece(pch, i):
        # row-block (pch*4+i) of xh_d = piece i of chunk pch, fully contiguous
        r = pch * 4 + i
        return xh_d.ap()[r * 128:(r + 1) * 128, :]

    def src3(i):
        return xpiece(3, i)
    # sync + gpsimd rings carry the big x/wb/owT stream; ALL small constants
    # go on the third (scalar/qActDynamicHW) ring so nothing delays x3/wb and
    # mt/xth land early (v3 had mt 5th on sync -> qproj(3) stalled 2.1us)
    # each kt-pair's weights ride the SAME ring immediately ahead of its x
    # piece, pairs split across the two fast rings -- so kt0/1 (sync) and
    # kt2/3 (gpsimd) both become computable at ~12.5us and kt4-7 at ~15.5,
    # instead of serializing every weight behind every x piece on one ring.
    # Small constants go on the slow scalar ring (needed only >=15us).
    def wb_piece(j):
        # wb_d row-block j holds kt-pair j's weights [128, 2*C] contiguous
        return wb_d.ap()[j * 128:(j + 1) * 128, :].rearrange(
            "p (a b) -> p a b", b=C)

    nc.sync.dma_start(out=wbA_sb[:, :2, :], in_=wb_piece(0))
    nc.gpsimd.dma_start(out=wbA_sb[:, 2:, :], in_=wb_piece(1))
    # kt0's x alone first: the first matmul can start ~1us earlier than
    # with a full 256KB kt0+kt1 piece at the slow early DMA rate
    nc.sync.dma_start(out=xt3[:, 0:512], in_=src3(0)[:, 0:512])
    nc.sync.dma_start(out=xt3[:, 512:1024], in_=src3(0)[:, 512:1024])
    nc.gpsimd.dma_start(out=xt3[:, 1024:2048], in_=src3(1))
    nc.scalar.dma_start(out=ident_sb, in_=ident_d.ap())
    nc.sync.dma_start(out=wbB_sb[:, :2, :], in_=wb_piece(2))
    nc.gpsimd.dma_start(out=wbB_sb[:, 2:, :], in_=wb_piece(3))
    nc.sync.dma_start(out=xt3[:, 2048:3072], in_=src3(2))
    nc.gpsimd.dma_start(out=xt3[:, 3072:4096], in_=src3(3))
    nc.scalar.dma_start(out=maskC_sb, in_=maskC_d.ap())
    nc.scalar.dma_start(out=mt_sb, in_=mt_d.ap().rearrange("p (a b) -> p a b", b=C))
    nc.gpsimd.dma_start(out=xth, in_=xhh_d.ap())

    def dma_x(pch, engs=None):
        xt = xpool.tile([128, KT * PCH], BF, tag="xt", name=f"x{pch}")
        engs = engs or [nc.sync, nc.gpsimd, nc.sync, nc.gpsimd]
        for i in range(4):
            engs[i].dma_start(out=xt[:, i * 1024:(i + 1) * 1024],
                              in_=xpiece(pch, i))
        return xt

    # ---- PE clock warmup: REAL matmuls (transpose-mode doesn't count as
    # PE activity for the HAM throttle).  The input is memset on DVE so the
    # warmup needs NO DMA -- it starts right after the framework preamble
    # (v3 waited 3.4us for the first DMA to land before any PE activity)
    warm_in = consts.tile([128, 128], BF)
    nc.vector.memset(warm_in, 0.0)
    warm_ps = psT.tile([128, 128], FP32, tag="pt", name="warm")

    def filler(n):
        for _ in range(n):
            nc.tensor.matmul(warm_ps, lhsT=warm_in, rhs=warm_in,
                             start=True, stop=True)

    filler(N_WARM)

    def outproj(pch, rTt, final=False):
        t0 = pch * PCH
        gsz = 4
        tag = "ogF" if final else "og"
        ogs = [ostage.tile([128, gsz, PCH], BF, tag=tag, name=tag)
               for _ in range(KT // gsz)]
        fengs = [nc.sync, nc.scalar]
        for vt in range(KT):
            ps = psO.tile([128, PCH], FP32, tag="po", name="ops")
            for ct in range(CT):
                nc.tensor.matmul(
                    ps, lhsT=owT_sb[:, ct * V + vt * 128:ct * V + (vt + 1) * 128],
                    rhs=rTt[:, ct, :], start=(ct == 0), stop=(ct == CT - 1))
            cp = nc.vector.tensor_copy if vt % 2 == 0 else nc.scalar.copy
            cp(ogs[vt // gsz][:, vt % gsz, :], ps)
            if vt % gsz == gsz - 1:
                g0 = vt - gsz + 1
                if final:
                    eng = fengs[vt // gsz]
                else:
                    eng = nc.sync if (vt // gsz) % 2 == 0 else nc.gpsimd
                eng.dma_start(
                    out=outT_d.ap()[g0 * 128:(vt + 1) * 128,
                                    t0:t0 + PCH].rearrange("(a p) t -> p a t",
                                                           p=128),
                    in_=ogs[vt // gsz])

    prev = {}
    xts = {3: xt3}
    for idx, p in enumerate(range(N_PCH - 1, -1, -1)):
        # ---- prefetch next x chunk (and owT once, needed by outproj(3))
        if idx == 0:
            xts[2] = dma_x(2)
            nc.sync.dma_start(out=owT_sb[:, :V], in_=owT_d.ap()[0:128, :])
            nc.gpsimd.dma_start(out=owT_sb[:, V:], in_=owT_d.ap()[128:256, :])
            xts[1] = dma_x(1)
        elif idx == 1:
            xts[0] = dma_x(0)

        # ---- inproj(p): xbT[c, t] = wb^T @ x
        xt = xts.pop(p)
        kTt = kpool.tile([128, CT, PCH], BF, tag="kT")
        if idx == 0:
            # DMA-paced: kt-major so each arriving x piece gates only its
            # own matmuls; fillers keep the HAM activity window hot while
            # the x pieces stream in (PE would idle anyway), padded heavier
            # right before the piece boundaries
            pcs = [psA.tile([128, PCH], FP32, tag="ps", name="xps")
                   for _ in range(CT)]
            for kt in range(KT):
                wbt = wbA_sb if kt < 4 else wbB_sb
                for ct in range(CT):
                    nc.tensor.matmul(
                        pcs[ct], lhsT=wbt[:, kt % 4, ct * 128:(ct + 1) * 128],
                        rhs=xt[:, kt * PCH:(kt + 1) * PCH],
                        start=(kt == 0), stop=(kt == KT - 1))
                if kt < 7:
                    filler({0: 4, 3: 12, 5: 8}.get(kt, 2))
            nc.scalar.copy(kTt[:, 0, :], pcs[0])
            nc.vector.tensor_copy(kTt[:, 1, :], pcs[1])
        else:
            # ct-major; ct0 in one full psum (stops mid-section, evacuates
            # during ct1), ct1 in two half psums so the last piece of kT
            # lands ~0.35us after the final MM -- qproj's cp1 then stalls
            # ~0.3us instead of ~0.8 (one full ct1 psum evac is ~0.69us)
            pc0 = psA.tile([128, PCH], FP32, tag="ps", name="xps")
            for kt in range(KT):
                wbt = wbA_sb if kt < 4 else wbB_sb
                nc.tensor.matmul(
                    pc0, lhsT=wbt[:, kt % 4, 0:128],
                    rhs=xt[:, kt * PCH:(kt + 1) * PCH],
                    start=(kt == 0), stop=(kt == KT - 1))
            nc.scalar.copy(kTt[:, 0, :], pc0)
            for tb in range(2):
                pc1 = psA.tile([128, PCH // 2], FP32, tag="ps", name="xps")
                for kt in range(KT):
                    wbt = wbA_sb if kt < 4 else wbB_sb
                    nc.tensor.matmul(
                        pc1, lhsT=wbt[:, kt % 4, 128:256],
                        rhs=xt[:, kt * PCH + tb * 256:kt * PCH + (tb + 1) * 256],
                        start=(kt == 0), stop=(kt == KT - 1))
                cp_ = nc.vector.tensor_copy if tb == 0 else nc.scalar.copy
                cp_(kTt[:, 1, tb * 256:(tb + 1) * 256], pc1)

        # ---- qproj(p): q~T = M^T @ xbT.  Right after inproj: the following
        # outproj block fully hides the qT evacuation latency before A needs
        # it (v5 measured 0.5-1.3us A/intra stalls with qproj late)
        qTt = qpool.tile([128, CT, PCH], BF, tag="qT")
        for ct in range(CT):
            qps = psO.tile([128, PCH], FP32, tag="po", name="qps")
            for cp in range(CT):
                nc.tensor.matmul(
                    qps, lhsT=mt_sb[:, cp, ct * 128:(ct + 1) * 128],
                    rhs=kTt[:, cp, :], start=(cp == 0), stop=(cp == CT - 1))
            cp_ = nc.scalar.copy if ct == 0 else nc.vector.tensor_copy
            cp_(qTt[:, ct, :], qps)

        if 0 < idx < N_PCH - 1:
            outproj(p + 1, prev["rT"])

        # ---- transposes(p): v = xb in [t, c] layout
        vvt = vpool.tile([128, TPP, C], BF, tag="v")
        for tb in range(TPP):
            pst = psT.tile([128, C], BF, tag="pt", name="pst")
            for ct in range(CT):
                nc.tensor.transpose(
                    pst[:, ct * 128:(ct + 1) * 128],
                    kTt[:, ct, tb * 128:(tb + 1) * 128], ident_sb)
            cp_ = nc.vector.tensor_copy if tb % 2 == 0 else nc.scalar.copy
            cp_(vvt[:, tb, :], pst)
        if idx == 0:
            # halo inproj + transpose, deferred here so xth has extra
            # time to land
            hps = [psT.tile([128, HALO], FP32, tag="pt", name="hps")
                   for _ in range(CT)]
            for kt in range(KT):
                wbt = wbA_sb if kt < 4 else wbB_sb
                for ct in range(CT):
                    nc.tensor.matmul(
                        hps[ct], lhsT=wbt[:, kt % 4, ct * 128:(ct + 1) * 128],
                        rhs=xth[:, kt * HALO:(kt + 1) * HALO],
                        start=(kt == 0), stop=(kt == KT - 1))
            nc.scalar.copy(kTh_sb[:, 0, :], hps[0])
            nc.scalar.copy(kTh_sb[:, 1, :], hps[1])
            psth = psT.tile([128, C], BF, tag="pt", name="pst")
            for ct in range(CT):
                nc.tensor.transpose(
                    psth[:, ct * 128:(ct + 1) * 128], kTh_sb[:, ct, :],
                    ident_sb)
            nc.scalar.copy(vh_sb, psth)

        # ---- A blocks: s-block j = 4p+rel; A[s_rel, t] = k_s . q~_t, masked.
        # rel 1..3 serve two t-tiles at once (256 cols: d1 for tile j-1 +
        # d0 for tile j); rel 0 = d0-only edge, rel 4 = d1-only edge (next
        # pch / halo s-block).
        atms = [None] * 5
        for rel in ((0, 1, 2, 3, 4) if idx == 0 else (4, 0, 1, 2, 3)):
            if rel == 0:
                w, q0, msk = 128, 0, maskC_sb[:, 128:]
            elif rel == 4:
                w, q0, msk = 128, 3 * 128, maskC_sb[:, :128]
            else:
                w, q0, msk = 256, (rel - 1) * 128, maskC_sb
            aps = psT.tile([128, w], FP32, tag="pt", name="aps",
                           padded_shape=[128, 256])
            for ct in range(CT):
                if rel < 4:
                    lhs = kTt[:, ct, rel * 128:(rel + 1) * 128]
                elif idx == 0:
                    lhs = kTh_sb[:, ct, :]
                else:
                    lhs = prev["kT"][:, ct, 0:128]
                nc.tensor.matmul(aps, lhsT=lhs, rhs=qTt[:, ct, q0:q0 + w],
                                 start=(ct == 0), stop=(ct == CT - 1))
            am = apool.tile([128, w], BF, tag="atm", name="atm",
                            padded_shape=[128, 256])
            nc.vector.tensor_mul(am, aps, msk)
            atms[rel] = am

        # ---- intra: rT[c, t] += v_j^T @ atm_j (banded accumulation).
        # start=True on the first matmul clears has_written for the whole
        # bank, so the staggered sub-region writes overwrite-then-accumulate
        # correctly.
        rTt = rpool.tile([128, CT, PCH], BF, tag="rT")
        for ct in range(CT):
            ips = psT.tile([128, PCH], FP32, tag="pt", name="ips")
            if idx == 0:
                v4 = vh_sb[:, ct * 128:(ct + 1) * 128]
            else:
                v4 = prev["v"][:, 0, ct * 128:(ct + 1) * 128]
            segs = [
                (0, 128, atms[0], vvt[:, 0, ct * 128:(ct + 1) * 128]),
                (0, 256, atms[1], vvt[:, 1, ct * 128:(ct + 1) * 128]),
                (128, 256, atms[2], vvt[:, 2, ct * 128:(ct + 1) * 128]),
                (256, 256, atms[3], vvt[:, 3, ct * 128:(ct + 1) * 128]),
                (384, 128, atms[4], v4),
            ]
            for si, (c0, w, am, lhs) in enumerate(segs):
                nc.tensor.matmul(ips[:, c0:c0 + w], lhsT=lhs, rhs=am,
                                 start=(si == 0), stop=(si == len(segs) - 1))
            nc.scalar.copy(rTt[:, ct, :], ips)

        if idx == N_PCH - 1:
            # deferred: fills the PE while intra(0)'s rT evacuations land,
            # so the epilogue outproj(0) doesn't stall on them
            outproj(p + 1, prev["rT"])
        prev = {"kT": kTt, "v": vvt, "rT": rTt}

    outproj(0, prev["rT"], final=True)

    ctx.close()


# ---------------- host side ----------------

_NC_CACHE = None


def _get_nc():
    global _NC_CACHE
    if _NC_CACHE is None:
        _NC_CACHE = build_nc()
    return _NC_CACHE


def _tile128(a, inner):
    """[G*128, inner] -> [128, G*inner] with block g at columns [g*inner,...)."""
    g = a.shape[0] // 128
    return np.ascontiguousarray(
        a.reshape(g, 128, inner).transpose(1, 0, 2).reshape(128, g * inner))


def _prep_in_maps(inputs):
    x = np.asarray(inputs["x"], np.float32)
    basis = np.asarray(inputs["basis"], np.float32)
    decay = float(1.0 / (1.0 + np.exp(-np.float64(inputs["decay_logit"]))))
    out_scale = float(np.float32(inputs["out_scale"]))

    qc = np.asarray(inputs["q_coeffs"], np.float32)
    kc = np.asarray(inputs["k_coeffs"], np.float32)
    vc = np.asarray(inputs["v_coeffs"], np.float32)
    oc = np.asarray(inputs["o_coeffs"], np.float32)

    wb0 = _tile128(basis.astype(BF16), C)                      # [128, KT*C]
    # piece-major: row-block j = kt-pair j, [128, 2*C] contiguous per piece
    wb = np.ascontiguousarray(
        wb0.reshape(128, 4, 2 * C).transpose(1, 0, 2)).reshape(4 * 128, 2 * C)
    mt = _tile128((qc.T @ kc).astype(BF16), C)                 # [128, CT*C]
    owT0 = _tile128(((vc.T @ oc) @ basis.T * out_scale).astype(BF16), V)
    owT = np.ascontiguousarray(
        owT0.reshape(128, 2, V).transpose(1, 0, 2)).reshape(2 * 128, V)

    sr = np.arange(128, dtype=np.float64)[:, None]
    tr = np.arange(128, dtype=np.float64)[None, :]
    m1 = decay ** (127.0 + sr - tr)                        # d1: j = i+1 block
    m0 = np.where(sr > tr, decay ** np.maximum(sr - tr - 1.0, 0.0), 0.0)
    maskC = np.concatenate([m1, m0], axis=1).astype(np.float32)
    ident = np.eye(128, dtype=np.float32).astype(BF16)

    T_EXT = T_LOC + HALO
    in_maps = []
    for core in range(N_CORES):
        b, h = divmod(core, 2)
        t0 = h * T_LOC
        te = min(t0 + T_EXT, T)
        xT = np.zeros((V, T_EXT), dtype=BF16)
        xT[:, :te - t0] = x[b, t0:te].T.astype(BF16)
        # pre-tile main: [pch][p][kt][t] contiguous; halo: [p][kt][t]
        # piece-major: row-block (pch*4+i) = [128, 1024] piece, contiguous.
        # column j of a piece = kt (2i..2i+1) block col: piece cols =
        # [kt=2i|kt=2i+1] each 512 t-cols of chunk pch
        xh0 = np.ascontiguousarray(
            xT[:, :T_LOC].reshape(KT, 128, N_PCH, PCH).transpose(2, 1, 0, 3)
        ).reshape(N_PCH, 128, KT * PCH)
        xh = np.ascontiguousarray(
            xh0.reshape(N_PCH, 128, 4, 1024).transpose(0, 2, 1, 3)
        ).reshape(N_PCH * 4 * 128, 1024)
        xhh = np.ascontiguousarray(
            xT[:, T_LOC:].reshape(KT, 128, HALO).transpose(1, 0, 2)
        ).reshape(128, KT * HALO)
        in_maps.append({
            "xh": xh, "xhh": xhh, "wb": wb, "mt": mt, "owT": owT,
            "maskC": maskC, "ident": ident,
        })
    return in_maps


def _ensure_ntff_hook():
    """The agent image's antenv package lacks axon_hooks; shim it so
    run_bass_kernel_spmd(trace=True) can register the NTFF profile hook."""
    try:
        from antenv.axon_hooks import get_axon_ntff_profile_hook  # noqa: F401
        return
    except ImportError:
        pass
    import sys
    import types
    import antenv
    mod = types.ModuleType("antenv.axon_hooks")
    _state = {"hook": None}
    mod.set_axon_ntff_profile_hook = lambda h: _state.__setitem__("hook", h)
    mod.get_axon_ntff_profile_hook = lambda: _state["hook"]
    sys.modules["antenv.axon_hooks"] = mod
    antenv.axon_hooks = mod
    from trn_agent_boot.trn_boot import _ntff_profile_via_ctypes
    mod.set_axon_ntff_profile_hook(
        _ntff_profile_via_ctypes("/opt/axon/libaxon_pjrt.so"))


def run(inputs, trace=False):
    """Returns (out [B,T,V] float32, BassKernelResults)."""
    if trace:
        _ensure_ntff_hook()
    in_maps = _prep_in_maps(inputs)
    nc = _get_nc()
    res = run_bass_kernel_spmd(nc, in_maps, core_ids=list(range(N_CORES)),
                               trace=trace)
    out = np.zeros((B, T, V), np.float32)
    for core in range(N_CORES):
        b, h = divmod(core, 2)
        outT = np.asarray(res.results[core]["outT"]).astype(np.float32)
        out[b, h * T_LOC:(h + 1) * T_LOC] = outT.T
    return out, res


def kernel(**inputs):
    out, _ = run(inputs, trace=False)
    return out


# revision 22
# speedup vs baseline: 1.0136x; 1.0136x over previous
"""Trainium2 Bass kernel for nn_AssociativeMemoryStep (forward-looking retention).

reference semantics:
    q,k,v,o weights = basis @ {q,k,v,o}_coeffs.T          [V, C]
    q/k/v = x @ w                                         [B, T, C]
    scores[t,s] = (q_t . k_s) * decay^(s-t-1) for s>t     (anti-causal)
    retrieved = scores @ v ; out = retrieved @ o_w.T * out_scale

Factored (basis-space) formulation: all four weights share the factor
`basis` [V, 2NB], so with xb = x @ basis [T, 2NB]:
    scores    = (xb @ M) @ xb^T          M  = q_coeffs^T @ k_coeffs  [2NB,2NB]
    out       = (scores_decayed @ xb) @ Wo'
    Wo'       = v_coeffs^T @ oc @ basis^T * out_scale               [2NB, V]
i.e. retention runs with q := xb@M, k := v := xb.

BANDED retention: decay = sigmoid(3) ~ 0.9526, so contributions from
s - t > 256 are < 4e-6.  Each 128-wide t-tile i attends only s-blocks
j = i (d0, strict lower-tri decay mask) and j = i+1 (d1, full decay
mask); truncation rel-l2 ~ 6e-4.  No recurrent state, no cross-chunk
terms -- all tiles fully independent, which lets retention + output
projection interleave with the DMA-paced projection phase.

Sharding: 8 cores = 4 batches x 2 sequence halves of T_LOC=2048, each
with a HALO=128 slice of the following positions recomputed locally.

Schedule: reverse pch (512-col) order; iter p emits
  inproj(p) -> qproj(p) -> outproj(p+1) -> transposes(p) ->
  [halo @ first iter] -> A-blocks(p) -> intra(p)
(outproj of the previous chunk hides the kT/qT evacuation latency; the
last iter defers outproj(1) behind intra(0) to hide the rT(0) evacs
before the epilogue outproj(0)).

Measured constraints this schedule is built around:
- HAM clock gate: PE runs at 1.2GHz until ~3.4us of SUSTAINED matmul
  activity; transpose-mode does NOT count.  Warmup = real matmuls on a
  memset tile (no DMA dependency, starts right after the preamble) and
  fillers pad the DMA-paced first inproj so the window never breaks.
- DMA: issues start only after the ~6.5us framework preamble; first
  bytes land ~8.3us; ~300-360GB/s once ramped.  Input is 5.7MB, so the
  head of the stream carries exactly what the first matmuls need (each
  kt-pair's weights ahead of its x piece, on the same ring; big x/og
  traffic on sync+gpsimd, small constants on the scalar ring), and
  every piece is a fully contiguous DRAM block (host pre-tiled).
- PSUM evacuation is ~690ns per [128,512] fp32 bank on either ACT or
  DVE (1x mode): evacuations alternate engines everywhere, and outproj
  og evacuations alternate per-vt so neither engine falls behind the
  0.43us/psum matmul pace.
- Each HWDGE store pays ~1.5-2us completion latency: the final output
  goes out as 2 large 512KB stores (one per HWDGE ring), not 8 small.
"""

import contextlib

import numpy as np
import ml_dtypes

import concourse.bass as bass
import concourse.mybir as mybir
import concourse.tile as tile
from concourse import bacc
from concourse.bass_utils import run_bass_kernel_spmd

BF16 = ml_dtypes.bfloat16

B, T, V, C = 4, 4096, 1024, 256   # C = 2*N_BASIS = basis channels
N_CORES = 8
T_LOC = 2048          # main positions per core
HALO = 128            # halo positions beyond T_LOC (d1 context for tile 15)
PCH = 512             # projection t-chunk
N_PCH = T_LOC // PCH  # 4
KT = V // 128         # 8 v-ktiles
CT = C // 128         # 2 c-tiles
TPP = PCH // 128      # 4 t-tiles per pch
N_WARM = 52           # warmup matmuls bridging [preamble-end, x3p0 ~12.5us];
                      # ~107ns each at the cold 1.2GHz clock

FP32 = mybir.dt.float32
BF = mybir.dt.bfloat16


def build_nc():
    nc = bacc.Bacc("TRN2", target_bir_lowering=False, debug=False,
                   num_devices=N_CORES)

    # all inputs host-pre-tiled to be contiguous per [128, N] DMA block
    xh_d = nc.dram_tensor("xh", [N_PCH * 4 * 128, 1024], BF, kind="ExternalInput")
    xhh_d = nc.dram_tensor("xhh", [128, KT * HALO], BF, kind="ExternalInput")
    wb_d = nc.dram_tensor("wb", [4 * 128, 2 * C], BF, kind="ExternalInput")
    mt_d = nc.dram_tensor("mt", [128, CT * C], BF, kind="ExternalInput")
    owT_d = nc.dram_tensor("owT", [2 * 128, V], BF, kind="ExternalInput")
    maskC_d = nc.dram_tensor("maskC", [128, 2 * 128], FP32, kind="ExternalInput")
    ident_d = nc.dram_tensor("ident", [128, 128], BF, kind="ExternalInput")
    outT_d = nc.dram_tensor("outT", [V, T_LOC], BF, kind="ExternalOutput")

    with tile.TileContext(nc) as tc:
        build_tile(tc, xh_d, xhh_d, wb_d, mt_d, owT_d, maskC_d, ident_d, outT_d)
    nc.compile()
    return nc


def build_tile(tc, xh_d, xhh_d, wb_d, mt_d, owT_d, maskC_d, ident_d, outT_d):
    nc = tc.nc

    ctx = contextlib.ExitStack()
    consts = ctx.enter_context(tc.tile_pool(name="consts", bufs=1))
    xpool = ctx.enter_context(tc.tile_pool(name="xpool", bufs=4))
    kpool = ctx.enter_context(tc.tile_pool(name="kpool", bufs=2))
    qpool = ctx.enter_context(tc.tile_pool(name="qpool", bufs=2))
    vpool = ctx.enter_context(tc.tile_pool(name="vpool", bufs=2))
    rpool = ctx.enter_context(tc.tile_pool(name="rpool", bufs=2))
    apool = ctx.enter_context(tc.tile_pool(name="apool", bufs=6))
    ostage = ctx.enter_context(tc.tile_pool(name="ostage", bufs=4))
    psA = ctx.enter_context(tc.tile_pool(name="psA", bufs=2, space="PSUM"))
    psO = ctx.enter_context(tc.tile_pool(name="psO", bufs=3, space="PSUM"))
    psT = ctx.enter_context(tc.tile_pool(name="psT", bufs=3, space="PSUM"))

    # ---- constant tiles ----
    wbA_sb = consts.tile([128, 4, C], BF)
    wbB_sb = consts.tile([128, 4, C], BF)
    mt_sb = consts.tile([128, CT, C], BF)
    owT_sb = consts.tile([128, CT * V], BF)
    maskC_sb = consts.tile([128, 2 * 128], FP32)   # [d1 mask | d0 tri mask]
    ident_sb = consts.tile([128, 128], BF)
    kTh_sb = consts.tile([128, CT, HALO], BF)      # halo xbT
    vh_sb = consts.tile([128, C], BF)              # halo xb in [t, c]
    xth = consts.tile([128, KT * HALO], BF)        # halo x staging

    # ---- startup DMAs; HWDGE(sync) + SWDGE(gpsimd) queues run concurrently,
    # pieces ordered so wb[kt]+x3[kt] needed by the first matmuls land first
    xt3 = xpool.tile([128, KT * PCH], BF, tag="xt", name="x3")

    def xpiece(pch, i):
        # row-block (pch*4+i) of xh_d = piece i of chunk pch, fully contiguous
        r = pch * 4 + i
        return xh_d.ap()[r * 128:(r + 1) * 128, :]

    def src3(i):
        return xpiece(3, i)
    # sync + gpsimd rings carry the big x/wb/owT stream; ALL small constants
    # go on the third (scalar/qActDynamicHW) ring so nothing delays x3/wb and
    # mt/xth land early (v3 had mt 5th on sync -> qproj(3) stalled 2.1us)
    # each kt-pair's weights ride the SAME ring immediately ahead of its x
    # piece, pairs split across the two fast rings -- so kt0/1 (sync) and
    # kt2/3 (gpsimd) both become computable at ~12.5us and kt4-7 at ~15.5,
    # instead of serializing every weight behind every x piece on one ring.
    # Small constants go on the slow scalar ring (needed only >=15us).
    def wb_piece(j):
        # wb_d row-block j holds kt-pair j's weights [128, 2*C] contiguous
        return wb_d.ap()[j * 128:(j + 1) * 128, :].rearrange(
            "p (a b) -> p a b", b=C)

    nc.sync.dma_start(out=wbA_sb[:, :2, :], in_=wb_piece(0))
    nc.gpsimd.dma_start(out=wbA_sb[:, 2:, :], in_=wb_piece(1))
    # kt0's x alone first: the first matmul can start ~1us earlier than
    # with a full 256KB kt0+kt1 piece at the slow early DMA rate
    nc.sync.dma_start(out=xt3[:, 0:512], in_=src3(0)[:, 0:512])
    nc.sync.dma_start(out=xt3[:, 512:1024], in_=src3(0)[:, 512:1024])
    nc.gpsimd.dma_start(out=xt3[:, 1024:2048], in_=src3(1))
    nc.scalar.dma_start(out=ident_sb, in_=ident_d.ap())
    nc.sync.dma_start(out=wbB_sb[:, :2, :], in_=wb_piece(2))
    nc.gpsimd.dma_start(out=wbB_sb[:, 2:, :], in_=wb_piece(3))
    nc.sync.dma_start(out=xt3[:, 2048:3072], in_=src3(2))
    nc.gpsimd.dma_start(out=xt3[:, 3072:4096], in_=src3(3))
    nc.scalar.dma_start(out=maskC_sb, in_=maskC_d.ap())
    nc.scalar.dma_start(out=mt_sb, in_=mt_d.ap().rearrange("p (a b) -> p a b", b=C))
    nc.gpsimd.dma_start(out=xth, in_=xhh_d.ap())

    def dma_x(pch, engs=None):
        xt = xpool.tile([128, KT * PCH], BF, tag="xt", name=f"x{pch}")
        engs = engs or [nc.sync, nc.gpsimd, nc.sync, nc.gpsimd]
        for i in range(4):
            engs[i].dma_start(out=xt[:, i * 1024:(i + 1) * 1024],
                              in_=xpiece(pch, i))
        return xt

    # ---- PE clock warmup: REAL matmuls (transpose-mode doesn't count as
    # PE activity for the HAM throttle).  The input is memset on DVE so the
    # warmup needs NO DMA -- it starts right after the framework preamble
    # (v3 waited 3.4us for the first DMA to land before any PE activity)
    warm_in = consts.tile([128, 128], BF)
    nc.vector.memset(warm_in, 0.0)
    warm_ps = psT.tile([128, 128], FP32, tag="pt", name="warm")

    def filler(n):
        for _ in range(n):
            nc.tensor.matmul(warm_ps, lhsT=warm_in, rhs=warm_in,
                             start=True, stop=True)

    filler(N_WARM)

    def outproj(pch, rTt, final=False):
        t0 = pch * PCH
        gsz = 4
        tag = "ogF" if final else "og"
        ogs = [ostage.tile([128, gsz, PCH], BF, tag=tag, name=tag)
               for _ in range(KT // gsz)]
        fengs = [nc.sync, nc.scalar]
        for vt in range(KT):
            ps = psO.tile([128, PCH], FP32, tag="po", name="ops")
            for ct in range(CT):
                nc.tensor.matmul(
                    ps, lhsT=owT_sb[:, ct * V + vt * 128:ct * V + (vt + 1) * 128],
                    rhs=rTt[:, ct, :], start=(ct == 0), stop=(ct == CT - 1))
            cp = nc.vector.tensor_copy if vt % 2 == 0 else nc.scalar.copy
            cp(ogs[vt // gsz][:, vt % gsz, :], ps)
            if vt % gsz == gsz - 1:
                g0 = vt - gsz + 1
                if final:
                    eng = fengs[vt // gsz]
                else:
                    eng = nc.sync if (vt // gsz) % 2 == 0 else nc.scalar
                eng.dma_start(
                    out=outT_d.ap()[g0 * 128:(vt + 1) * 128,
                                    t0:t0 + PCH].rearrange("(a p) t -> p a t",
                                                           p=128),
                    in_=ogs[vt // gsz])

    prev = {}
    xts = {3: xt3}
    for idx, p in enumerate(range(N_PCH - 1, -1, -1)):
        # ---- prefetch next x chunk (and owT once, needed by outproj(3))
        if idx == 0:
            xts[2] = dma_x(2)
            nc.sync.dma_start(out=owT_sb[:, :V], in_=owT_d.ap()[0:128, :])
            nc.gpsimd.dma_start(out=owT_sb[:, V:], in_=owT_d.ap()[128:256, :])
            xts[1] = dma_x(1)
        elif idx == 1:
            xts[0] = dma_x(0)

        # ---- inproj(p): xbT[c, t] = wb^T @ x
        xt = xts.pop(p)
        kTt = kpool.tile([128, CT, PCH], BF, tag="kT")
        if idx == 0:
            # DMA-paced: kt-major so each arriving x piece gates only its
            # own matmuls; fillers keep the HAM activity window hot while
            # the x pieces stream in (PE would idle anyway), padded heavier
            # right before the piece boundaries
            pcs = [psA.tile([128, PCH], FP32, tag="ps", name="xps")
                   for _ in range(CT)]
            for kt in range(KT):
                wbt = wbA_sb if kt < 4 else wbB_sb
                for ct in range(CT):
                    nc.tensor.matmul(
                        pcs[ct], lhsT=wbt[:, kt % 4, ct * 128:(ct + 1) * 128],
                        rhs=xt[:, kt * PCH:(kt + 1) * PCH],
                        start=(kt == 0), stop=(kt == KT - 1))
                if kt < 7:
                    filler({0: 4, 3: 12, 5: 8}.get(kt, 2))
            nc.scalar.copy(kTt[:, 0, :], pcs[0])
            nc.vector.tensor_copy(kTt[:, 1, :], pcs[1])
        else:
            # ct-major; ct0 in one full psum (stops mid-section, evacuates
            # during ct1), ct1 in two half psums so the last piece of kT
            # lands ~0.35us after the final MM -- qproj's cp1 then stalls
            # ~0.3us instead of ~0.8 (one full ct1 psum evac is ~0.69us)
            pc0 = psA.tile([128, PCH], FP32, tag="ps", name="xps")
            for kt in range(KT):
                wbt = wbA_sb if kt < 4 else wbB_sb
                nc.tensor.matmul(
                    pc0, lhsT=wbt[:, kt % 4, 0:128],
                    rhs=xt[:, kt * PCH:(kt + 1) * PCH],
                    start=(kt == 0), stop=(kt == KT - 1))
            nc.scalar.copy(kTt[:, 0, :], pc0)
            for tb in range(2):
                pc1 = psA.tile([128, PCH // 2], FP32, tag="ps", name="xps")
                for kt in range(KT):
                    wbt = wbA_sb if kt < 4 else wbB_sb
                    nc.tensor.matmul(
                        pc1, lhsT=wbt[:, kt % 4, 128:256],
                        rhs=xt[:, kt * PCH + tb * 256:kt * PCH + (tb + 1) * 256],
                        start=(kt == 0), stop=(kt == KT - 1))
                cp_ = nc.vector.tensor_copy if tb == 0 else nc.scalar.copy
                cp_(kTt[:, 1, tb * 256:(tb + 1) * 256], pc1)

        if idx == 0:
            # halo inproj here: independent PE work covering the kT(3)
            # evacuation latency before qproj(3) reads it
            hps = [psT.tile([128, HALO], FP32, tag="pt", name="hps")
                   for _ in range(CT)]
            for kt in range(KT):
                wbt = wbA_sb if kt < 4 else wbB_sb
                for ct in range(CT):
                    nc.tensor.matmul(
                        hps[ct], lhsT=wbt[:, kt % 4, ct * 128:(ct + 1) * 128],
                        rhs=xth[:, kt * HALO:(kt + 1) * HALO],
                        start=(kt == 0), stop=(kt == KT - 1))
            nc.scalar.copy(kTh_sb[:, 0, :], hps[0])
            nc.scalar.copy(kTh_sb[:, 1, :], hps[1])

        # ---- qproj(p): q~T = M^T @ xbT.  Right after inproj: the following
        # outproj block fully hides the qT evacuation latency before A needs
        # it (v5 measured 0.5-1.3us A/intra stalls with qproj late)
        qTt = qpool.tile([128, CT, PCH], BF, tag="qT")
        for ct in range(CT):
            qps = psO.tile([128, PCH], FP32, tag="po", name="qps")
            for cp in range(CT):
                nc.tensor.matmul(
                    qps, lhsT=mt_sb[:, cp, ct * 128:(ct + 1) * 128],
                    rhs=kTt[:, cp, :], start=(cp == 0), stop=(cp == CT - 1))
            cp_ = nc.scalar.copy if ct == 0 else nc.vector.tensor_copy
            cp_(qTt[:, ct, :], qps)

        if 0 < idx < N_PCH - 1:
            outproj(p + 1, prev["rT"])

        # ---- transposes(p): v = xb in [t, c] layout
        vvt = vpool.tile([128, TPP, C], BF, tag="v")
        for tb in range(TPP):
            pst = psT.tile([128, C], BF, tag="pt", name="pst")
            for ct in range(CT):
                nc.tensor.transpose(
                    pst[:, ct * 128:(ct + 1) * 128],
                    kTt[:, ct, tb * 128:(tb + 1) * 128], ident_sb)
            cp_ = nc.vector.tensor_copy if tb % 2 == 0 else nc.scalar.copy
            cp_(vvt[:, tb, :], pst)
        if idx == 0:
            psth = psT.tile([128, C], BF, tag="pt", name="pst")
            for ct in range(CT):
                nc.tensor.transpose(
                    psth[:, ct * 128:(ct + 1) * 128], kTh_sb[:, ct, :],
                    ident_sb)
            nc.scalar.copy(vh_sb, psth)

        # ---- A blocks: s-block j = 4p+rel; A[s_rel, t] = k_s . q~_t, masked.
        # rel 1..3 serve two t-tiles at once (256 cols: d1 for tile j-1 +
        # d0 for tile j); rel 0 = d0-only edge, rel 4 = d1-only edge (next
        # pch / halo s-block).
        atms = [None] * 5
        for rel in ((0, 1, 2, 3, 4) if idx == 0 else (4, 0, 1, 2, 3)):
            if rel == 0:
                w, q0, msk = 128, 0, maskC_sb[:, 128:]
            elif rel == 4:
                w, q0, msk = 128, 3 * 128, maskC_sb[:, :128]
            else:
                w, q0, msk = 256, (rel - 1) * 128, maskC_sb
            aps = psT.tile([128, w], FP32, tag="pt", name="aps",
                           padded_shape=[128, 256])
            for ct in range(CT):
                if rel < 4:
                    lhs = kTt[:, ct, rel * 128:(rel + 1) * 128]
                elif idx == 0:
                    lhs = kTh_sb[:, ct, :]
                else:
                    lhs = prev["kT"][:, ct, 0:128]
                nc.tensor.matmul(aps, lhsT=lhs, rhs=qTt[:, ct, q0:q0 + w],
                                 start=(ct == 0), stop=(ct == CT - 1))
            am = apool.tile([128, w], BF, tag="atm", name="atm",
                            padded_shape=[128, 256])
            nc.vector.tensor_mul(am, aps, msk)
            atms[rel] = am

        # ---- intra: rT[c, t] += v_j^T @ atm_j (banded accumulation).
        # start=True on the first matmul clears has_written for the whole
        # bank, so the staggered sub-region writes overwrite-then-accumulate
        # correctly.
        rTt = rpool.tile([128, CT, PCH], BF, tag="rT")
        for ct in range(CT):
            ips = psT.tile([128, PCH], FP32, tag="pt", name="ips")
            if idx == 0:
                v4 = vh_sb[:, ct * 128:(ct + 1) * 128]
            else:
                v4 = prev["v"][:, 0, ct * 128:(ct + 1) * 128]
            segs = [
                (0, 128, atms[0], vvt[:, 0, ct * 128:(ct + 1) * 128]),
                (0, 256, atms[1], vvt[:, 1, ct * 128:(ct + 1) * 128]),
                (128, 256, atms[2], vvt[:, 2, ct * 128:(ct + 1) * 128]),
                (256, 256, atms[3], vvt[:, 3, ct * 128:(ct + 1) * 128]),
                (384, 128, atms[4], v4),
            ]
            for si, (c0, w, am, lhs) in enumerate(segs):
                nc.tensor.matmul(ips[:, c0:c0 + w], lhsT=lhs, rhs=am,
                                 start=(si == 0), stop=(si == len(segs) - 1))
            nc.scalar.copy(rTt[:, ct, :], ips)

        if idx == N_PCH - 1:
            # deferred: fills the PE while intra(0)'s rT evacuations land,
            # so the epilogue outproj(0) doesn't stall on them
            outproj(p + 1, prev["rT"])
        prev = {"kT": kTt, "v": vvt, "rT": rTt}

    outproj(0, prev["rT"], final=True)

    ctx.close()


# ---------------- host side ----------------

_NC_CACHE = None


def _get_nc():
    global _NC_CACHE
    if _NC_CACHE is None:
        _NC_CACHE = build_nc()
    return _NC_CACHE


def _tile128(a, inner):
    """[G*128, inner] -> [128, G*inner] with block g at columns [g*inner,...)."""
    g = a.shape[0] // 128
    return np.ascontiguousarray(
        a.reshape(g, 128, inner).transpose(1, 0, 2).reshape(128, g * inner))


def _prep_in_maps(inputs):
    x = np.asarray(inputs["x"], np.float32)
    basis = np.asarray(inputs["basis"], np.float32)
    decay = float(1.0 / (1.0 + np.exp(-np.float64(inputs["decay_logit"]))))
    out_scale = float(np.float32(inputs["out_scale"]))

    qc = np.asarray(inputs["q_coeffs"], np.float32)
    kc = np.asarray(inputs["k_coeffs"], np.float32)
    vc = np.asarray(inputs["v_coeffs"], np.float32)
    oc = np.asarray(inputs["o_coeffs"], np.float32)

    wb0 = _tile128(basis.astype(BF16), C)                      # [128, KT*C]
    # piece-major: row-block j = kt-pair j, [128, 2*C] contiguous per piece
    wb = np.ascontiguousarray(
        wb0.reshape(128, 4, 2 * C).transpose(1, 0, 2)).reshape(4 * 128, 2 * C)
    mt = _tile128((qc.T @ kc).astype(BF16), C)                 # [128, CT*C]
    owT0 = _tile128(((vc.T @ oc) @ basis.T * out_scale).astype(BF16), V)
    owT = np.ascontiguousarray(
        owT0.reshape(128, 2, V).transpose(1, 0, 2)).reshape(2 * 128, V)

    sr = np.arange(128, dtype=np.float64)[:, None]
    tr = np.arange(128, dtype=np.float64)[None, :]
    m1 = decay ** (127.0 + sr - tr)                        # d1: j = i+1 block
    m0 = np.where(sr > tr, decay ** np.maximum(sr - tr - 1.0, 0.0), 0.0)
    maskC = np.concatenate([m1, m0], axis=1).astype(np.float32)
    ident = np.eye(128, dtype=np.float32).astype(BF16)

    T_EXT = T_LOC + HALO
    in_maps = []
    for core in range(N_CORES):
        b, h = divmod(core, 2)
        t0 = h * T_LOC
        te = min(t0 + T_EXT, T)
        xT = np.zeros((V, T_EXT), dtype=BF16)
        xT[:, :te - t0] = x[b, t0:te].T.astype(BF16)
        # pre-tile main: [pch][p][kt][t] contiguous; halo: [p][kt][t]
        # piece-major: row-block (pch*4+i) = [128, 1024] piece, contiguous.
        # column j of a piece = kt (2i..2i+1) block col: piece cols =
        # [kt=2i|kt=2i+1] each 512 t-cols of chunk pch
        xh0 = np.ascontiguousarray(
            xT[:, :T_LOC].reshape(KT, 128, N_PCH, PCH).transpose(2, 1, 0, 3)
        ).reshape(N_PCH, 128, KT * PCH)
        xh = np.ascontiguousarray(
            xh0.reshape(N_PCH, 128, 4, 1024).transpose(0, 2, 1, 3)
        ).reshape(N_PCH * 4 * 128, 1024)
        xhh = np.ascontiguousarray(
            xT[:, T_LOC:].reshape(KT, 128, HALO).transpose(1, 0, 2)
        ).reshape(128, KT * HALO)
        in_maps.append({
            "xh": xh, "xhh": xhh, "wb": wb, "mt": mt, "owT": owT,
            "maskC": maskC, "ident": ident,
        })
    return in_maps


def _ensure_ntff_hook():
    """The agent image's antenv package lacks axon_hooks; shim it so
    run_bass_kernel_spmd(trace=True) can register the NTFF profile hook."""
    try:
        from antenv.axon_hooks import get_axon_ntff_profile_hook  # noqa: F401
        return
    except ImportError:
        pass
    import sys
    import types
    import antenv
    mod = types.ModuleType("antenv.axon_hooks")
    _state = {"hook": None}
    mod.set_axon_ntff_profile_hook = lambda h: _state.__setitem__("hook", h)
    mod.get_axon_ntff_profile_hook = lambda: _state["hook"]
    sys.modules["antenv.axon_hooks"] = mod
    antenv.axon_hooks = mod
    from trn_agent_boot.trn_boot import _ntff_profile_via_ctypes
    mod.set_axon_ntff_profile_hook(
        _ntff_profile_via_ctypes("/opt/axon/libaxon_pjrt.so"))


def run(inputs, trace=False):
    """Returns (out [B,T,V] float32, BassKernelResults)."""
    if trace:
        _ensure_ntff_hook()
    in_maps = _prep_in_maps(inputs)
    nc = _get_nc()
    res = run_bass_kernel_spmd(nc, in_maps, core_ids=list(range(N_CORES)),
                               trace=trace)
    out = np.zeros((B, T, V), np.float32)
    for core in range(N_CORES):
        b, h = divmod(core, 2)
        outT = np.asarray(res.results[core]["outT"]).astype(np.float32)
        out[b, h * T_LOC:(h + 1) * T_LOC] = outT.T
    return out, res


def kernel(**inputs):
    out, _ = run(inputs, trace=False)
    return out
